# revision 1
# baseline (speedup 1.0000x reference)
"""Trainium2 Bass kernel for the token-scan problem.

Math: the reference scans T=128 tokens updating (x, rho) and emits
concat([x_T, y_T, v*_T, rho_T.ravel()]).  Because the x-recurrence depends
only on the (known) token sequence, the whole scan unrolls into dense
matmuls:

  V   = token_emb[tokens]                [T, d]
  R   = relu(Dx @ V^T)                   [n, T]
  X   = cumsum(R, axis=1)                [n, T]   (X[:,i] = x_i)
  g   = X^T @ x_f                        [T]      (x_f = X[:, T-1])
  a   = V^T @ (w * g),  w_j = c^(T-1-j) (j<T-1)   [d]  == rho_{T-2} @ x_{T-1}
  y   = relu(Dy @ ln(a)) * relu(x_f)     [n]
  v*  = ln(E @ y)                        [d]
  rho = (V * w')^T @ X^T, w'_j = c^(T-j) [d, n]

Sharding: n split across 8 cores (Dx/Dy rows, E columns, rho columns, x/y
slices).  Cross-core comm: one AllReduce of g [T] and one of E@y [d].
"""

import numpy as np

N, D, V_VOCAB, T = 16384, 256, 32000, 128
DECAY = 0.97
EPS = 1e-6
N_CORES = 8
NS = N // N_CORES           # 2048 rows per core
NQ = NS // 512              # 4 free-dim chunks of 512
NT = NS // 128              # 16 tiles of 128

_cache = {}
STAGE = 99   # debug: how much of the program to emit


def _build():
    stage = STAGE
    import concourse.bacc as bacc
    import concourse.mybir as mybir
    import concourse.tile as tile

    dt = mybir.dt.float32
    AF = mybir.ActivationFunctionType
    ALU = mybir.AluOpType

    nc = bacc.Bacc("TRN2", target_bir_lowering=False, debug=False,
                   num_devices=N_CORES)

    # Per-core inputs (already laid out for SBUF: 128 partitions first).
    # consts packs [vts | u | v | vwp | wcol] into one tensor -> one DMA.
    i_dxts = nc.dram_tensor("dxts", [128, 2 * NS], dt, kind="ExternalInput")
    i_dyts = nc.dram_tensor("dyts", [128, 2 * NS], dt, kind="ExternalInput")
    i_ets = nc.dram_tensor("ets", [128, NT * 256], dt, kind="ExternalInput")
    i_consts = nc.dram_tensor("consts", [128, 897], dt, kind="ExternalInput")

    o_x = nc.dram_tensor("out_x", [NS], dt, kind="ExternalOutput")
    o_y = nc.dram_tensor("out_y", [NS], dt, kind="ExternalOutput")
    o_vs = nc.dram_tensor("out_vs", [256], dt, kind="ExternalOutput")
    o_rho = nc.dram_tensor("out_rho", [256, NS], dt, kind="ExternalOutput")

    with tile.TileContext(nc) as tc:
        with (
            tc.tile_pool(name="persist", bufs=1) as pp,
            tc.tile_pool(name="work", bufs=2) as wp,
            tc.tile_pool(name="psA", bufs=3, space="PSUM") as psA,
            tc.tile_pool(name="psS", bufs=1, space="PSUM") as psS,
            tc.tile_pool(name="psG", bufs=1, space="PSUM") as psG,
            tc.tile_pool(name="psR", bufs=1, space="PSUM") as psR,
            tc.tile_pool(name="dram", bufs=1, space="DRAM") as dram,
        ):
            # ---- load constants / operands ----
            # dxts is on the critical path: split across the SP and Pool DMA
            # rings so the two halves transfer in parallel.
            consts = pp.tile([128, 897], dt)
            nc.sync.dma_start(consts[:], i_consts[:])
            dxts = pp.tile([128, 2 * NS], dt)
            nc.sync.dma_start(dxts[:, :NS], i_dxts[:, :NS])
            nc.gpsimd.dma_start(dxts[:, NS:], i_dxts[:, NS:])
            vts = consts[:, 0:256]
            u = consts[:, 256:384]
            v = consts[:, 384:640]
            vwp = consts[:, 640:896]
            wcol = consts[:, 896:897]
            dyts = pp.tile([128, 2 * NS], dt)
            ets = pp.tile([128, NT * 256], dt)

            ones_col = u[:, 127:128]   # [128, 1] of ones

            # ---- RT = relu(V @ Dx^T) : [T=128, n] ----
            rt = pp.tile([128, NS], dt)
            for q in range(NQ):
                rt_ps = psA.tile([128, 512], dt, tag="mmA")
                for c in range(2):
                    nc.tensor.matmul(
                        rt_ps[:],
                        lhsT=vts[:, c * 128:(c + 1) * 128],
                        rhs=dxts[:, c * NS + q * 512: c * NS + (q + 1) * 512],
                        start=(c == 0), stop=(c == 1),
                    )
                nc.scalar.activation(rt[:, q * 512:(q + 1) * 512], rt_ps[:],
                                     AF.Relu)

            if stage >= 4:
                # ---- g = X^T x_f = U^T h with h = R^T x_f ----
                # Rcol_i = relu(Dx_i @ V^T) in [n, T] layout straight from
                # dxts; the relu's accum_out emits x_f columns for free
                # (x_f >= 0 since it's a sum of relus).  h accumulates in two
                # alternating PSUM banks; g = U^T h is a cumsum matmul.
                xfcol = pp.tile([128, NT], dt)
                rcols = pp.tile([128, NT * 128], dt)
                for i in range(NT):
                    rc_ps = psA.tile([128, 128], dt, tag="mmA")
                    for c in range(2):
                        nc.tensor.matmul(
                            rc_ps[:],
                            lhsT=dxts[:, c * NS + i * 128:
                                      c * NS + (i + 1) * 128],
                            rhs=vts[:, c * 128:(c + 1) * 128],
                            start=(c == 0), stop=(c == 1))
                    nc.scalar.activation(rcols[:, i * 128:(i + 1) * 128],
                                         rc_ps[:], AF.Relu,
                                         accum_out=xfcol[:, i:i + 1])
                h_ps0 = psG.tile([128, 1], dt, tag="g0")
                h_ps1 = psG.tile([128, 1], dt, tag="g1")
                for i in range(NT):
                    nc.tensor.matmul((h_ps0 if i % 2 == 0 else h_ps1)[:],
                                     lhsT=rcols[:, i * 128:(i + 1) * 128],
                                     rhs=xfcol[:, i:i + 1],
                                     start=(i < 2), stop=(i >= NT - 2))
                h0 = pp.tile([128, 1], dt)
                nc.vector.tensor_copy(h0[:], h_ps0[:])
                h = pp.tile([128, 1], dt)
                nc.vector.tensor_add(h[:], h0[:], h_ps1[:])
                g_ps = psS.tile([128, 1], dt, tag="small")
                nc.tensor.matmul(g_ps[:], lhsT=u[:], rhs=h[:],
                                 start=True, stop=True)
                g = pp.tile([128, 1], dt)
                nc.vector.tensor_copy(g[:], g_ps[:])
                nc.sync.dma_start(o_x[:].rearrange("(i p) -> p i", p=128),
                                  xfcol[:])

            if stage >= 5:
                # ---- AllReduce g across cores ----
                # dyts/ets (needed only after the AllReduce) are queued on the
                # Pool ring just before the collective so they drain during it.
                nc.gpsimd.dma_start(dyts[:], i_dyts[:])
                nc.gpsimd.dma_start(ets[:], i_ets[:])
                g_in = dram.tile([128, 1], dt)
                g_out = dram.tile([128, 1], dt)
                nc.sync.dma_start(g_in[:], g[:])
                nc.gpsimd.collective_compute(
                    "AllReduce", ALU.add,
                    replica_groups=[list(range(N_CORES))],
                    ins=[g_in.opt()], outs=[g_out.opt()],
                )
                gfull = pp.tile([128, 1], dt)
                nc.sync.dma_start(gfull[:], g_out[:])

            if stage >= 6:
                # ---- rho = (V*w')^T @ XT : [256, n] (overlaps AllReduce) ----
                for dc in range(2):
                    rho_sb = wp.tile([128, NS], dt, tag="rho_sb")
                    for q in range(NQ):
                        rho_ps = psA.tile([128, 512], dt, tag="mmA")
                        nc.tensor.matmul(rho_ps[:],
                                         lhsT=vwp[:, dc * 128:(dc + 1) * 128],
                                         rhs=rt[:, q * 512:(q + 1) * 512],
                                         start=True, stop=True)
                        nc.vector.tensor_copy(
                            rho_sb[:, q * 512:(q + 1) * 512], rho_ps[:])
                    nc.sync.dma_start(o_rho[dc * 128:(dc + 1) * 128, :],
                                      rho_sb[:])

            def emit_ln(src_ap, out_sb, L):
                # (z - mean) / (std_unbiased + eps), per torch layernorm_row.
                k = emit_ln.k
                cp = pp.tile([1, L], dt, tag=f"ln_cp{k}")
                m = pp.tile([1, 1], dt, tag=f"ln_m{k}")
                # copy with scale 1/L; accum_out gives the mean directly
                nc.scalar.activation(cp[:], src_ap, AF.Copy, scale=1.0 / L,
                                     accum_out=m[:])
                cen = pp.tile([1, L], dt, tag=f"ln_c{k}")
                nc.vector.tensor_scalar_sub(cen[:], src_ap, m[:])
                sq = pp.tile([1, L], dt, tag=f"ln_q{k}")
                ssq = pp.tile([1, 1], dt, tag=f"ln_ss{k}")
                nc.scalar.activation(sq[:], cen[:], AF.Square,
                                     accum_out=ssq[:])
                std = pp.tile([1, 1], dt, tag=f"ln_sd{k}")
                nc.scalar.activation(std[:], ssq[:], AF.Sqrt,
                                     scale=1.0 / (L - 1))
                stde = pp.tile([1, 1], dt, tag=f"ln_se{k}")
                nc.vector.tensor_scalar_add(stde[:], std[:], EPS)
                inv = pp.tile([1, 1], dt, tag=f"ln_i{k}")
                nc.vector.reciprocal(inv[:], stde[:])
                nc.vector.tensor_scalar_mul(out_sb, cen[:], inv[:])
                emit_ln.k += 1

            emit_ln.k = 0

            if stage >= 7:
                # ---- a = (V*w)^T g : [1, 256] then layernorm ----
                # (decay weights w are folded into v host-side)
                a_ps = psR.tile([1, 256], dt, tag="row")
                nc.tensor.matmul(a_ps[:], lhsT=gfull[:], rhs=v[:],
                                 start=True, stop=True)
                aln = pp.tile([1, 256], dt)
                emit_ln(a_ps[:], aln[:], 256)

                # aln to column layout [128, 2] via SBUF->SBUF DMA
                alnc = pp.tile([128, 2], dt)
                for h in range(2):
                    nc.sync.dma_start(alnc[:, h:h + 1],
                                      aln[0:1, h * 128:(h + 1) * 128])

            if stage >= 8:
                # ---- ycore[:, i] = Dy_chunk_i @ aln; y = relu(yc)*relu(xf) ----
                yc_ps = psS.tile([128, NT], dt, tag="small")
                for i in range(NT):
                    for h in range(2):
                        nc.tensor.matmul(
                            yc_ps[:, i:i + 1],
                            lhsT=dyts[:, h * NS + i * 128:
                                      h * NS + (i + 1) * 128],
                            rhs=alnc[:, h:h + 1],
                            start=(h == 0), stop=(h == 1))
                ycr = pp.tile([128, NT], dt)
                nc.scalar.activation(ycr[:], yc_ps[:], AF.Relu)
                y = pp.tile([128, NT], dt)
                nc.vector.tensor_mul(y[:], ycr[:], xfcol[:])
                nc.sync.dma_start(o_y[:].rearrange("(i p) -> p i", p=128),
                                  y[:])

            if stage >= 9:
                # ---- vs partial = y^T @ E^T : [1, 256] ----
                vs_ps0 = psG.tile([1, 256], dt, tag="g0")
                vs_ps1 = psG.tile([1, 256], dt, tag="g1")
                for i in range(NT):
                    nc.tensor.matmul((vs_ps0 if i % 2 == 0 else vs_ps1)[:],
                                     lhsT=y[:, i:i + 1],
                                     rhs=ets[:, i * 256:(i + 1) * 256],
                                     start=(i < 2), stop=(i >= NT - 2))
                vsp0 = pp.tile([1, 256], dt)
                nc.vector.tensor_copy(vsp0[:], vs_ps0[:])
                vsp = pp.tile([1, 256], dt)
                nc.vector.tensor_add(vsp[:], vsp0[:], vs_ps1[:])

            if stage >= 10:
                vs_in = dram.tile([1, 256], dt)
                vs_out = dram.tile([1, 256], dt)
                nc.sync.dma_start(vs_in[:], vsp[:])
                nc.gpsimd.collective_compute(
                    "AllReduce", ALU.add,
                    replica_groups=[list(range(N_CORES))],
                    ins=[vs_in.opt()], outs=[vs_out.opt()],
                )
                vsf = pp.tile([1, 256], dt)
                nc.sync.dma_start(vsf[:], vs_out[:])
                vsln = pp.tile([1, 256], dt)
                emit_ln(vsf[:], vsln[:], 256)
                nc.sync.dma_start(o_vs[:].rearrange("(a b) -> a b", a=1),
                                  vsln[0:1, :])

    nc.finalize()
    return nc


def _host_prep(E, Dx, Dy, token_emb, tokens):
    E = np.asarray(E, dtype=np.float32)
    Dx = np.asarray(Dx, dtype=np.float32)
    Dy = np.asarray(Dy, dtype=np.float32)
    token_emb = np.asarray(token_emb, dtype=np.float32)
    tokens = np.asarray(tokens).astype(np.int64)

    v = np.ascontiguousarray(token_emb[tokens])          # [T, d]
    vts = np.concatenate([v[:, :128].T, v[:, 128:].T], axis=1)  # [128, 256]
    j = np.arange(T)
    w = (DECAY ** ((T - 1) - j)).astype(np.float32)
    w[T - 1] = 0.0
    wp = (DECAY ** (T - j)).astype(np.float32)
    u_host = np.triu(np.ones((T, T), dtype=np.float32))
    vwp = np.ascontiguousarray(
        (u_host @ (v * wp[:, None])).astype(np.float32))
    u = np.triu(np.ones((T, T), dtype=np.float32))
    wcol = w[:, None].astype(np.float32)
    vw = (v * w[:, None]).astype(np.float32)
    consts = np.ascontiguousarray(
        np.concatenate([vts, u, vw, vwp, wcol], axis=1).astype(np.float32))

    in_maps = []
    for k in range(N_CORES):
        sl = slice(k * NS, (k + 1) * NS)
        dx_s = Dx[sl]                                    # [NS, 256]
        dy_s = Dy[sl]
        e_s = E[:, sl]                                   # [256, NS]
        dxts = np.concatenate([dx_s[:, :128].T, dx_s[:, 128:].T], axis=1)
        dyts = np.concatenate([dy_s[:, :128].T, dy_s[:, 128:].T], axis=1)
        ets = np.concatenate(
            [e_s[:, i * 128:(i + 1) * 128].T for i in range(NT)], axis=1)
        in_maps.append({
            "dxts": np.ascontiguousarray(dxts),
            "dyts": np.ascontiguousarray(dyts),
            "ets": np.ascontiguousarray(ets),
            "consts": consts,
        })
    return in_maps


def kernel(E, Dx, Dy, token_emb, tokens, _trace=False):
    from concourse.bass_utils import run_bass_kernel_spmd

    key = ("nc", STAGE)
    if key not in _cache:
        _cache[key] = _build()
    nc = _cache[key]

    in_maps = _host_prep(E, Dx, Dy, token_emb, tokens)
    res = run_bass_kernel_spmd(nc, in_maps, core_ids=list(range(N_CORES)),
                               trace=_trace)
    _cache["last_result"] = res

    r = res.results
    x_full = np.concatenate([r[k]["out_x"] for k in range(N_CORES)])
    y_full = np.concatenate([r[k]["out_y"] for k in range(N_CORES)])
    vs = r[0]["out_vs"]
    rho = np.concatenate([r[k]["out_rho"] for k in range(N_CORES)], axis=1)
    return np.concatenate([x_full, y_full, vs, rho.ravel()]).astype(np.float32)

